# revision 2
# baseline (speedup 1.0000x reference)
"""Trainium2 Bass kernel for nn_CRF: 5 mean-field iterations of
y = x + w*blur(softmax(.)) on (16,384,384,21) f32, data-parallel over batch
across 8 NeuronCores (2 samples per core).

Self-contained: builds the Bass program, shards inputs, runs via
bass_utils.run_bass_kernel_spmd, reassembles the full output.
"""
import sys
if "/opt/trn_rl_repo" not in sys.path:
    sys.path.insert(0, "/opt/trn_rl_repo")

"""CRF mean-field kernel for TRN2 — shared builder used by test.py; kernel.py embeds a copy.

Algorithm (per sample, 5 iterations):
    xt = x                                  # (C,H,W) logits
    repeat 5: xt = x + w * blur(softmax(xt, axis=C))
Device formulation (free layout (c,w) per H-row partition):
    e0 = exp(x) (host, bf16), p = softmax(x) (host, bf16, iter-0 input)
    per iter: z = Bh-conv(p)  [transpose-style matmul, H-contraction]
              s = Bw-conv(z)  [transpose-style matmul, W-contraction]
              e = exp(s) * e0 ; p = e / sum_c(e)
    final iter: y = s + x (f32)
"""

import numpy as np
import ml_dtypes

H = W = 384
C = 21
FREE = C * W  # 8064
FS = 9
HALF = FS // 2  # 4
N_ITER = 5
SAMPLES_PER_CORE = 2
N_CORES = 8

# K-piece table: (src_tile, k0, k1, n0, n1)
# src partitions [k0_local .. k1_local) of tile, output band columns [n0, n1).
# k0 is 32-aligned (hardware base-partition constraint); band rows outside the
# 9-tap reach are zero.
# Order matters for PSUM per-element accumulate semantics: within each
# overlap region the wide (pending->overwrite) piece must come before the
# narrow accumulating one, so every matmul's written region is homogeneous.
PIECES = [
    (0, 0, 128, 0, 124),      # t0 main
    (1, 0, 128, 124, 244),    # t1 main
    (0, 0, 128, 124, 132),    # t0->t1 boundary (8 cols, rows 120-127 nonzero)
    (2, 0, 128, 244, 364),    # t2 main (cols 244-251 are zero rows -> writes 0)
    (1, 0, 128, 244, 260),    # t1 tail: [244,252) alone + [252,260) overlap with t2
    (2, 0, 128, 364, 384),    # t2 tail (20 cols)
]
NPAD = 124  # padded band columns in DRAM
# channel groups for PSUM banking (G channels x 512 f32 = G banks per tile)
import os as _os
GSIZE = int(_os.environ.get("CRF_G", "2"))
PSUM_BUFS = int(_os.environ.get("CRF_PSUM_BUFS", "4"))
ORDER = _os.environ.get("CRF_ORDER", "jout")  # gout | jout
ZSPLIT = int(_os.environ.get("CRF_ZSPLIT", "55"))      # % of z-copies on ACT
ESMUL = _os.environ.get("CRF_ESMUL", "dve_pool")        # dve | dve_pool | dve2_pool1
PMUL = _os.environ.get("CRF_PMUL", "dve")              # pool | dve
CGROUPS = [(c, min(c + GSIZE, C)) for c in range(0, C, GSIZE)]


def gauss_taps(inv_theta, spacing):
    d = spacing * np.arange(-HALF, HALF + 1, dtype=np.float64)
    k = np.exp(-((d * inv_theta) ** 2) / 2.0)
    k[HALF] = 0.0
    return k


def band_pieces(taps):
    """[6, 128, NPAD] f32 band matrices at absolute partition rows [k0,k1):
    band[p][k0+k, n] = taps[(k_abs - n_abs) + HALF].  (rhs must share the
    lhsT base partition, so piece data lives at its absolute rows.)"""
    out = np.zeros((len(PIECES), 128, NPAD), dtype=np.float64)
    for p, (t, k0, k1, n0, n1) in enumerate(PIECES):
        k_abs = t * 128 + np.arange(k0, k1)
        n_abs = np.arange(n0, n1)
        d = k_abs[:, None] - n_abs[None, :] + HALF
        m = (d >= 0) & (d < FS)
        out[p, k0:k1, : n1 - n0] = np.where(m, taps[np.clip(d, 0, FS - 1)], 0.0)
    return out.astype(np.float32)


def prep_inputs(x, spatial_spacings, smoothness_weight, inv_smoothness_theta):
    """Full inputs -> list of 8 per-core input dicts (host-side prep)."""
    x = np.asarray(x, dtype=np.float32)
    sp = np.asarray(spatial_spacings, dtype=np.float32)
    wgt = float(np.asarray(smoothness_weight))
    ith = np.asarray(inv_smoothness_theta, dtype=np.float32)

    B = x.shape[0]
    xt = np.ascontiguousarray(x.transpose(0, 1, 3, 2)).reshape(B, H, FREE)  # (B,H,(C,W))
    ef = np.exp(xt.reshape(B, H, C, W))
    s0 = ef.sum(axis=2, keepdims=True)
    p0 = (ef / s0).reshape(B, H, FREE).astype(ml_dtypes.bfloat16)
    e0 = ef.reshape(B, H, FREE).astype(ml_dtypes.bfloat16)

    in_maps = []
    prep_inputs.last_xt = xt  # stashed for unpack_outputs host-side add
    for core in range(N_CORES):
        bs = [core * SAMPLES_PER_CORE + i for i in range(SAMPLES_PER_CORE)]
        bh = np.stack([band_pieces(gauss_taps(ith[0], sp[b, 0])) for b in bs])
        bw = np.stack(
            [band_pieces(gauss_taps(ith[1], sp[b, 1])) * wgt for b in bs]
        )
        in_maps.append(
            {
                "p0": np.ascontiguousarray(p0[bs]),
                "e0": np.ascontiguousarray(e0[bs]),
                "bh": bh.astype(ml_dtypes.bfloat16),
                "bw": bw.astype(ml_dtypes.bfloat16),
            }
        )
    return in_maps


def unpack_outputs(results, xt=None):
    """list of per-core {'y': [2,H,FREE] bf16 s-values} -> full (16,H,W,C) f32.
    The final y = x + s add happens here on host in f32."""
    if xt is None:
        xt = prep_inputs.last_xt
    ss = np.concatenate([np.asarray(r["y"], dtype=np.float32) for r in results],
                        axis=0)  # (B, H, FREE)
    ys = xt[:ss.shape[0]] + ss
    return np.ascontiguousarray(
        ys.reshape(-1, H, C, W).transpose(0, 1, 3, 2)
    )  # (B,H,W,C)


def build_program(num_devices=N_CORES):
    import concourse.bacc as bacc
    import concourse.mybir as mybir
    import concourse.tile as tile

    f32 = mybir.dt.float32
    bf16 = mybir.dt.bfloat16
    AX = mybir.AxisListType
    AF = mybir.ActivationFunctionType

    nc = bacc.Bacc("TRN2", target_bir_lowering=False, debug=False,
                   num_devices=num_devices)

    S = SAMPLES_PER_CORE
    p0_d = nc.dram_tensor("p0", [S, H, FREE], bf16, kind="ExternalInput")
    e0_d = nc.dram_tensor("e0", [S, H, FREE], bf16, kind="ExternalInput")
    bh_d = nc.dram_tensor("bh", [S, 6, 128, NPAD], bf16, kind="ExternalInput")
    bw_d = nc.dram_tensor("bw", [S, 6, 128, NPAD], bf16, kind="ExternalInput")
    y_d = nc.dram_tensor("y", [S, H, FREE], bf16, kind="ExternalOutput")

    with tile.TileContext(nc) as tc:
        with (
            tc.tile_pool(name="res", bufs=1) as res,      # big residents
            tc.tile_pool(name="small", bufs=1) as small,  # bands, sums, rb
            tc.tile_pool(name="chunk", bufs=6) as chunk,  # es / xf / out staging
            tc.tile_pool(name="psum1", bufs=PSUM_BUFS, space="PSUM") as psum1,
        ):
            for b in range(S):
                # --- residents for this sample ---
                e_t = [res.tile([128, FREE], bf16, name=f"e{i}_s{b}", tag=f"e{i}")
                       for i in range(3)]
                z_t = [res.tile([128, FREE], bf16, name=f"z{i}_s{b}", tag=f"z{i}")
                       for i in range(3)]
                e0_t = [res.tile([128, FREE], bf16, name=f"e0{i}_s{b}", tag=f"e0{i}")
                        for i in range(3)]
                bh_t = [small.tile([128, NPAD], bf16, name=f"bh{p}_s{b}", tag=f"bh{p}")
                        for p in range(6)]
                bw_t = [small.tile([128, NPAD], bf16, name=f"bw{p}_s{b}", tag=f"bw{p}")
                        for p in range(6)]

                for p in range(6):
                    nc.sync.dma_start(bh_t[p][:], bh_d[b, p])
                    nc.sync.dma_start(bw_t[p][:], bw_d[b, p])
                ldq = [nc.sync, nc.scalar, nc.sync]
                for i in range(3):
                    ldq[i].dma_start(e_t[i][:], p0_d[b, 128 * i:128 * (i + 1), :])
                for i in range(3):
                    nc.gpsimd.dma_start(e0_t[i][:], e0_d[b, 128 * i:128 * (i + 1), :])

                for it in range(N_ITER):
                    # ---- stage 1: H-conv, e(=p) -> z ----
                    s1_iter = ([(c0c1, j) for c0c1 in CGROUPS for j in range(3)]
                               if ORDER == "gout" else
                               [(c0c1, j) for j in range(3) for c0c1 in CGROUPS])
                    for ((c0, c1), j) in s1_iter:
                        if True:
                            G = c1 - c0
                            ps = psum1.tile([128, GSIZE * 512], f32, name=f"ps1_{b}_{it}_{j}_{c0}",
                                           tag="ps1")
                            for ci, c in enumerate(range(c0, c1)):
                                npieces = len(PIECES)
                                for p, (t, k0, k1, n0, n1) in enumerate(PIECES):
                                    nc.tensor.matmul(
                                        ps[:, ci * 512 + n0: ci * 512 + n1],
                                        e_t[t][k0:k1, c * W + 128 * j: c * W + 128 * (j + 1)],
                                        bh_t[p][k0:k1, 0:n1 - n0],
                                        start=(p == 0),
                                        stop=(p == npieces - 1),
                                    )
                            # psum rows = w-chunk j ; cols = (c, h') -> z_j free (c,h')
                            zdst = z_t[j][:, c0 * W: c1 * W]
                            zsrc = ps.rearrange("p (c n) -> p c n", c=GSIZE)[:, 0:G, 0:W]
                            zk = (j * 11 + c0 // GSIZE) * 37 % 100
                            if zk < ZSPLIT:
                                nc.scalar.copy(zdst, zsrc)
                            else:
                                nc.vector.tensor_copy(zdst, zsrc)

                    # ---- stage 2: W-conv, z -> s ; update e or emit y ----
                    last = it == N_ITER - 1
                    s2_iter = ([(c0c1, i) for c0c1 in CGROUPS for i in range(3)]
                               if ORDER == "gout" else
                               [(c0c1, i) for i in range(3) for c0c1 in CGROUPS])
                    for ((c0, c1), i) in s2_iter:
                        if True:
                            G = c1 - c0
                            ps = psum1.tile([128, GSIZE * 512], f32, name=f"ps2_{b}_{it}_{i}_{c0}",
                                           tag="ps1")
                            for ci, c in enumerate(range(c0, c1)):
                                npieces = len(PIECES)
                                for p, (t, k0, k1, n0, n1) in enumerate(PIECES):
                                    nc.tensor.matmul(
                                        ps[:, ci * 512 + n0: ci * 512 + n1],
                                        z_t[t][k0:k1, c * W + 128 * i: c * W + 128 * (i + 1)],
                                        bw_t[p][k0:k1, 0:n1 - n0],
                                        start=(p == 0),
                                        stop=(p == npieces - 1),
                                    )
                            ps3 = ps.rearrange("p (c n) -> p c n", c=GSIZE)[:, 0:G, 0:W]
                            if not last:
                                es = chunk.tile([128, GSIZE * W], bf16,
                                                name=f"es_{b}_{it}_{i}_{c0}", tag="es")
                                nc.scalar.activation(
                                    es[:, 0:G * W].rearrange("p (c n) -> p c n", c=G),
                                    ps3, AF.Exp)
                                esk = (i + c0 // GSIZE)
                                if ESMUL == "dve":
                                    mul_eng = nc.vector
                                elif ESMUL == "dve_pool":
                                    mul_eng = nc.vector if esk % 2 == 0 else nc.gpsimd
                                else:  # dve2_pool1
                                    mul_eng = nc.vector if esk % 3 != 2 else nc.gpsimd
                                mul_eng.tensor_mul(
                                    e_t[i][:, c0 * W: c1 * W],
                                    es[:, 0:G * W],
                                    e0_t[i][:, c0 * W: c1 * W],
                                )
                            else:
                                yo = chunk.tile([128, GSIZE * W], bf16,
                                                name=f"yo_{b}_{it}_{i}_{c0}", tag="yo")
                                ydst = yo[:, 0:G * W].rearrange("p (c n) -> p c n", c=G)
                                if (i + c0) % 2 == 0:
                                    nc.scalar.copy(ydst, ps3)
                                else:
                                    nc.vector.tensor_copy(ydst, ps3)
                                nc.gpsimd.dma_start(
                                    y_d[b, 128 * i:128 * (i + 1), c0 * W: c1 * W],
                                    yo[:, 0:G * W])

                    if not last:
                        # ---- softmax normalize, w-chunked for pipelining:
                        # per (tile i, w-chunk j): sum over c (Pool tree-adds),
                        # rb = 1/sum (DVE), e *= rb (Pool), so stage-1 of the
                        # next iteration can start per chunk.
                        for i in range(3):
                            e3 = e_t[i].rearrange("p (c w) -> p c w", c=C)
                            for j in range(3):
                                wsl = slice(128 * j, 128 * (j + 1))
                                sm = small.tile([128, 128], f32,
                                                name=f"sm_{b}_{it}_{i}_{j}",
                                                tag=f"sm{i}{j}")
                                if j == 1:
                                    nc.vector.reduce_sum(
                                        sm[:],
                                        e_t[i].rearrange("p (c w) -> p w c", c=C)[:, wsl, :],
                                        axis=AX.X)
                                else:
                                    nc.gpsimd.tensor_add(
                                        sm[:], e3[:, 0, wsl], e3[:, 1, wsl])
                                    for c in range(2, C):
                                        nc.gpsimd.tensor_add(
                                            sm[:], sm[:], e3[:, c, wsl])
                                rb = small.tile([128, 128], bf16,
                                                name=f"rb_{b}_{it}_{i}_{j}",
                                                tag=f"rb{i}{j}")
                                with nc.allow_low_precision("1/sumexp in bf16"):
                                    nc.vector.reciprocal(rb[:], sm[:])
                                pm_eng = nc.gpsimd if PMUL == "pool" else nc.vector
                                pm_eng.tensor_mul(
                                    e3[:, :, wsl], e3[:, :, wsl],
                                    rb.unsqueeze(1).to_broadcast([128, C, 128]),
                                )

    nc.compile()
    return nc


def kernel(x, spatial_spacings, smoothness_weight, inv_smoothness_theta):
    import sys
    if "/opt/trn_rl_repo" not in sys.path:
        sys.path.insert(0, "/opt/trn_rl_repo")
    from concourse.bass_utils import run_bass_kernel_spmd

    in_maps = prep_inputs(x, spatial_spacings, smoothness_weight,
                          inv_smoothness_theta)
    nc = build_program()
    res = run_bass_kernel_spmd(nc, in_maps, core_ids=list(range(N_CORES)))
    return unpack_outputs(res.results)
